# revision 1
# baseline (speedup 1.0000x reference)
"""Multi-head self-attention (RoPE) Trainium2 Bass kernel.

Shards batch (B=8) across 8 NeuronCores, one batch element per core.
Per core: fused qkv projection (fp16 matmuls), RoPE, flash-style attention
(scores row-tiled per head pair, exp on ACT with fused scale+mask-bias,
AV row-tiled by k-parity with a fused ones-column producing softmax
denominators), reciprocal-normalize, output projection.
"""
import os
import sys

# The kernel needs the 8 axon-tunneled NeuronCores visible to jax; a
# JAX_PLATFORMS=cpu pin (used by some harnesses for the reference) would
# hide them. Clear it before jax initializes through the concourse imports.
os.environ.pop("JAX_PLATFORMS", None)

sys.path.insert(0, "/opt/trn_rl_repo")

_REPS = int(os.environ.get("KREPS", "1"))
_PH = int(os.environ.get("KPHASES", "3"))

import numpy as np
from contextlib import ExitStack

import concourse.bass as bass
import concourse.tile as tile
from concourse import bacc, mybir
from concourse.tile import add_dep_helper

f32 = mybir.dt.float32
f16 = mybir.dt.float16
AF = mybir.ActivationFunctionType
ALU = mybir.AluOpType

B, L, DIM = 8, 1024, 512
NH, HD = 8, 64
SCALE = HD ** -0.5
NCORES = 8


def _build_nc():
    nc = bacc.Bacc("TRN2", target_bir_lowering=False, debug=False, enable_asserts=False)

    xT = nc.dram_tensor("xT", (DIM, L), f16, kind="ExternalInput")
    wq = nc.dram_tensor("wq", (DIM, 2 * DIM), f16, kind="ExternalInput")  # Q|K cols
    wv = nc.dram_tensor("wv", (DIM, DIM), f16, kind="ExternalInput")      # V cols
    wp = nc.dram_tensor("wp", (DIM, DIM), f16, kind="ExternalInput")
    cosT = nc.dram_tensor("cosT", (128, L), f16, kind="ExternalInput")
    sinT = nc.dram_tensor("sinT", (128, L), f16, kind="ExternalInput")
    bias = nc.dram_tensor("bias", (128, 8), f32, kind="ExternalInput")
    y = nc.dram_tensor("y", (L, DIM), f32, kind="ExternalOutput")

    with ExitStack() as ctx:
        tc = ctx.enter_context(tile.TileContext(nc))
        cst = ctx.enter_context(tc.tile_pool(name="cst", bufs=1))
        sc = ctx.enter_context(tc.tile_pool(name="sc", bufs=3))
        pTp = ctx.enter_context(tc.tile_pool(name="pTp", bufs=10))
        nrm = ctx.enter_context(tc.tile_pool(name="nrm", bufs=2))
        ysb = ctx.enter_context(tc.tile_pool(name="ysb", bufs=1))

        # ---- load inputs ----
        xT_all = cst.tile([128, 4 * L], f16, name="t", tag="xTall")
        wq_all = cst.tile([128, 4 * 2 * DIM], f16, name="t", tag="wqall")
        wv_all = cst.tile([128, 4 * DIM], f16, name="t", tag="wvall")
        wp_all = cst.tile([128, 4 * DIM], f16, name="t", tag="wpall")
        for big, dram, w in ((xT_all, xT, L), (wq_all, wq, 2 * DIM),
                             (wv_all, wv, DIM), (wp_all, wp, DIM)):
            nc.sync.dma_start(
                big[:].rearrange("p (kc w) -> p kc w", kc=4),
                dram[:].rearrange("(kc p) w -> p kc w", p=128))
        xT_sb = [xT_all[:, i * L:(i + 1) * L] for i in range(4)]
        wq_sb = [wq_all[:, i * 2 * DIM:(i + 1) * 2 * DIM] for i in range(4)]
        wv_sb = [wv_all[:, i * DIM:(i + 1) * DIM] for i in range(4)]
        wp_sb = [wp_all[:, i * DIM:(i + 1) * DIM] for i in range(4)]
        cos_sb = cst.tile([128, L], f16, name="t", tag="cos")
        sin_sb = cst.tile([128, L], f16, name="t", tag="sin")
        bias_sb = cst.tile([128, 8], f32, name="t", tag="bias")
        nc.sync.dma_start(cos_sb[:], cosT[:])
        nc.sync.dma_start(sin_sb[:], sinT[:])
        nc.sync.dma_start(bias_sb[:], bias[:])

        qkT = [cst.tile([64, L], f16, name="t", tag=f"qkT{m}") for m in range(16)]
        vaug = [cst.tile([128, NH * 128], f16, name="t", tag=f"vaug{i}") for i in range(8)]
        outT = [cst.tile([128, L], f16, name="t", tag=f"outT{c}") for c in range(4)]

        def emit_body(rep):
            untiled1 = []
            tiled = []
            # ---------- phase 1: qkv projection + RoPE ----------
            with tc.tile_pool(name=f"qkps{rep}", bufs=2, space="PSUM") as qk_ps, \
                 tc.tile_pool(name=f"vps{rep}", bufs=2, space="PSUM") as v_ps:
                for m in range(8):
                    ps = qk_ps.tile([128, L], f32, name="t", tag="qkps")
                    for kc in range(4):
                        for qb in range(2):
                            mm = nc.tensor.matmul(
                                ps[:, qb * 512:(qb + 1) * 512],
                                wq_sb[kc][:, m * 128:(m + 1) * 128],
                                xT_sb[kc][:, qb * 512:(qb + 1) * 512],
                                start=(kc == 0), stop=(kc == 3))
                            untiled1.append(mm)
                    qc = sc.tile([128, L], f16, name="t", tag="qc")
                    nc.vector.tensor_copy(qc[:], ps[:])
                    sw = sc.tile([128, L], f16, name="t", tag="sw")
                    for (do, so) in ((0, 32), (32, 0), (64, 96), (96, 64)):
                        nc.vector.tensor_copy(sw[do:do + 32, :], qc[so:so + 32, :])
                    q1 = sc.tile([128, L], f16, name="t", tag="q1")
                    nc.vector.tensor_mul(q1[:], qc[:], cos_sb[:])
                    q2 = sc.tile([128, L], f16, name="t", tag="q2")
                    nc.vector.tensor_mul(q2[:], sw[:], sin_sb[:])
                    nc.vector.tensor_add(qkT[2 * m][:], q1[0:64, :], q2[0:64, :])
                    nc.vector.tensor_add(qkT[2 * m + 1][:], q1[64:128, :], q2[64:128, :])

                for lb in range(8):
                    vps = v_ps.tile([128, DIM], f32, name="t", tag="vps")
                    for kc in range(4):
                        mm = nc.tensor.matmul(
                            vps[:],
                            xT_sb[kc][:, lb * 128:(lb + 1) * 128],
                            wv_sb[kc][:],
                            start=(kc == 0), stop=(kc == 3))
                        untiled1.append(mm)
                    ones_ap = vaug[lb][:].rearrange("p (h c) -> p h c", h=NH)[:, :, 64:128]
                    nc.vector.memset(ones_ap, 1.0)
                    out_ap = vaug[lb][:].rearrange("p (h c) -> p h c", h=NH)[:, :, 0:64]
                    in_ap = vps[:].rearrange("p (h c) -> p h c", h=NH)
                    nc.vector.tensor_copy(out_ap, in_ap)

            # ---------- phase 2: attention (row-tiled 64x128) ----------
            with tc.tile_pool(name=f"sps{rep}", bufs=1, space="PSUM") as s_ps, \
                 tc.tile_pool(name=f"avps{rep}", bufs=2, space="PSUM") as av_ps:

                def emit_scores(p):
                    pts = {}
                    for t in range(2):
                        h = 2 * p + t
                        QTc, KTc = qkT[h], qkT[8 + h]
                        for kb2 in range(4):
                            s = s_ps.tile([128, 2048], f32, name="t", tag="s")
                            for half in range(2):
                                kb = kb2 * 2 + half
                                for qb in range(2):
                                    mm = nc.tensor.matmul(
                                        s[:, half * 1024 + qb * 512:
                                           half * 1024 + (qb + 1) * 512],
                                        KTc[:, kb * 128:(kb + 1) * 128],
                                        QTc[:, qb * 512:(qb + 1) * 512],
                                        start=True, stop=True)
                                    tiled.append(mm)
                            pt = pTp.tile([128, 2048], f16, name="t", tag="pT")
                            nc.scalar.activation(pt[:], s[:], AF.Exp,
                                                 bias=bias_sb[:, 2 * kb2:2 * kb2 + 1],
                                                 scale=SCALE)
                            pts[(t, kb2)] = pt
                    return pts

                def emit_av_norm(p, pts):
                    for t in range(2):
                        h = 2 * p + t
                        X = av_ps.tile([128, L], f32, name="t", tag="avX")
                        for kc in range(8):
                            pt = pts[(t, kc // 2)]
                            off = (kc % 2) * 1024
                            va = vaug[kc][:, h * 128:(h + 1) * 128]
                            for qb in range(2):
                                q0, q1_ = off + qb * 512, off + (qb + 1) * 512
                                mm = nc.tensor.matmul(
                                    X[:, qb * 512:(qb + 1) * 512], va, pt[:, q0:q1_],
                                    start=(kc == 0), stop=(kc == 7))
                                tiled.append(mm)
                        D = nrm.tile([64, L], f32, name="t", tag="D")
                        nc.vector.tensor_copy(D[:], X[64:128, :])
                        R = nrm.tile([64, L], f32, name="t", tag="R")
                        nc.vector.reciprocal_approx_fast(R[:], D[:])
                        if t == 0:
                            nc.vector.tensor_mul(outT[p][0:64, :], X[0:64, :], R[:])
                        else:
                            tmp = nrm.tile([64, L], f16, name="t", tag="tmp")
                            nc.vector.tensor_mul(tmp[:], X[0:64, :], R[:])
                            nc.sync.dma_start(outT[p][64:128, :], tmp[:])

                prev = None
                for p in range(4 if _PH >= 2 else 0):
                    pts = emit_scores(p)
                    if prev is not None and _PH >= 3:
                        emit_av_norm(prev[0], prev[1])
                    prev = (p, pts)
                if prev is not None and _PH >= 3:
                    emit_av_norm(prev[0], prev[1])

            # ---------- phase 3: output projection ----------
            with tc.tile_pool(name=f"yps{rep}", bufs=2, space="PSUM") as y_ps:
                yall = ysb.tile([128, 8 * DIM], f32, name="t", tag="yall")
                for lb in range(8 if _PH >= 3 else 0):
                    yp = y_ps.tile([128, DIM], f32, name="t", tag="yps")
                    for c in range(4):
                        mm = nc.tensor.matmul(
                            yp[:],
                            outT[c][:, lb * 128:(lb + 1) * 128],
                            wp_sb[c][:],
                            start=(c == 0), stop=(c == 3))
                    nc.vector.tensor_copy(yall[:, lb * DIM:(lb + 1) * DIM], yp[:])
                nc.sync.dma_start(
                    y[:].rearrange("(lb p) d -> p lb d", p=128),
                    yall[:].rearrange("p (lb d) -> p lb d", lb=8))

        for rep in range(_REPS):
            emit_body(rep)

    nc.compile()
    return nc


def _rope_tables():
    inv_freq = 1.0 / (10000.0 ** (np.arange(0, HD, 2, dtype=np.float32) / HD))
    t = np.arange(L, dtype=np.float32)
    freqs = np.outer(t, inv_freq)                      # (L, 32)
    emb = np.concatenate([freqs, freqs], axis=-1)      # (L, 64)
    cos = np.cos(emb).T                                # (64, L)
    sin = np.sin(emb).T                                # (64, L)
    sign = np.where(np.arange(HD) < HD // 2, -1.0, 1.0)[:, None].astype(np.float32)
    sin_s = sin * sign
    cosT = np.tile(cos, (2, 1)).astype(np.float16)     # (128, L)
    sinT = np.tile(sin_s, (2, 1)).astype(np.float16)   # (128, L)
    return cosT, sinT


_NC = None


def _get_nc():
    global _NC
    if _NC is None:
        _NC = _build_nc()
    return _NC


def kernel(x, mask, w_qkv, w_proj):
    x = np.asarray(x, dtype=np.float32)
    mask = np.asarray(mask)
    w_qkv = np.asarray(w_qkv, dtype=np.float32)
    w_proj = np.asarray(w_proj, dtype=np.float32)

    nc = _get_nc()
    cosT, sinT = _rope_tables()

    wq = np.ascontiguousarray(w_qkv[:, :2 * DIM]).astype(np.float16)
    wv = np.ascontiguousarray(w_qkv[:, 2 * DIM:]).astype(np.float16)
    wp = w_proj.astype(np.float16)

    in_maps = []
    for b in range(NCORES):
        xTb = np.ascontiguousarray(x[b].T).astype(np.float16)      # (512, 1024)
        bias_b = np.where(mask[b].reshape(8, 128).T, 0.0, -1e9).astype(np.float32)
        in_maps.append({
            "xT": xTb, "wq": wq, "wv": wv, "wp": wp,
            "cosT": cosT, "sinT": sinT, "bias": bias_b,
        })

    from concourse.bass_utils import run_bass_kernel_spmd
    res = run_bass_kernel_spmd(nc, in_maps, core_ids=list(range(NCORES)))
    out = np.stack([res.results[c]["y"] for c in range(NCORES)], axis=0)
    return out.astype(np.float32)



# revision 2
# speedup vs baseline: 27.0077x; 27.0077x over previous
"""Multi-head self-attention (RoPE) Trainium2 Bass kernel.

Shards batch (B=8) across 8 NeuronCores, one batch element per core.
Per core: fused qkv projection (fp16 matmuls), RoPE, flash-style attention
(scores row-tiled per head pair, exp on ACT with fused scale+mask-bias,
AV row-tiled by k-parity with a fused ones-column producing softmax
denominators), reciprocal-normalize, output projection.

The repetition count (_REPS) used for timing runs as a HARDWARE loop
(tc.For_i) on-device, so timing via rep differencing measures actual
device execution rather than host-side NEFF handling.
"""
import os
import sys

# The kernel needs the 8 axon-tunneled NeuronCores visible to jax; a
# JAX_PLATFORMS=cpu pin (used by some harnesses for the reference) would
# hide them. Clear it before jax initializes through the concourse imports.
os.environ.pop("JAX_PLATFORMS", None)

sys.path.insert(0, "/opt/trn_rl_repo")

_REPS = int(os.environ.get("KREPS", "1"))
_PH = int(os.environ.get("KPHASES", "3"))

import numpy as np
from contextlib import ExitStack

import concourse.bass as bass
import concourse.tile as tile
from concourse import bacc, mybir

f32 = mybir.dt.float32
f16 = mybir.dt.float16
AF = mybir.ActivationFunctionType
ALU = mybir.AluOpType

B, L, DIM = 8, 1024, 512
NH, HD = 8, 64
SCALE = HD ** -0.5
NCORES = 8


def _build_nc():
    nc = bacc.Bacc("TRN2", target_bir_lowering=False, debug=False, enable_asserts=False)

    xT = nc.dram_tensor("xT", (DIM, L), f16, kind="ExternalInput")
    wq = nc.dram_tensor("wq", (DIM, 2 * DIM), f16, kind="ExternalInput")  # Q|K cols
    wv = nc.dram_tensor("wv", (DIM, DIM), f16, kind="ExternalInput")      # V cols
    wp = nc.dram_tensor("wp", (DIM, DIM), f16, kind="ExternalInput")
    cosT = nc.dram_tensor("cosT", (128, L), f16, kind="ExternalInput")
    sinT = nc.dram_tensor("sinT", (128, L), f16, kind="ExternalInput")
    bias = nc.dram_tensor("bias", (128, 8), f32, kind="ExternalInput")
    y = nc.dram_tensor("y", (L, DIM), f32, kind="ExternalOutput")

    with ExitStack() as ctx:
        tc = ctx.enter_context(tile.TileContext(nc))
        cst = ctx.enter_context(tc.tile_pool(name="cst", bufs=1))
        sc = ctx.enter_context(tc.tile_pool(name="sc", bufs=3))
        pTp = ctx.enter_context(tc.tile_pool(name="pTp", bufs=10))
        nrm = ctx.enter_context(tc.tile_pool(name="nrm", bufs=2))
        ysb = ctx.enter_context(tc.tile_pool(name="ysb", bufs=1))
        # PSUM budget is 8 banks (2KB each per partition).  One pool holds
        # the 4-bank score tiles (tag "s", bufs=1); a second pool holds
        # 2-bank 128x1024 tiles (tag "b", bufs=2) shared by qkv projection,
        # AV accumulation, and the output projection.
        ps_s = ctx.enter_context(tc.tile_pool(name="pss", bufs=1, space="PSUM"))
        ps_b = ctx.enter_context(tc.tile_pool(name="psb", bufs=2, space="PSUM"))

        # ---- load inputs ----
        xT_all = cst.tile([128, 4 * L], f16, name="t", tag="xTall")
        wq_all = cst.tile([128, 4 * 2 * DIM], f16, name="t", tag="wqall")
        wv_all = cst.tile([128, 4 * DIM], f16, name="t", tag="wvall")
        wp_all = cst.tile([128, 4 * DIM], f16, name="t", tag="wpall")
        for big, dram, w in ((xT_all, xT, L), (wq_all, wq, 2 * DIM),
                             (wv_all, wv, DIM), (wp_all, wp, DIM)):
            nc.sync.dma_start(
                big[:].rearrange("p (kc w) -> p kc w", kc=4),
                dram[:].rearrange("(kc p) w -> p kc w", p=128))
        xT_sb = [xT_all[:, i * L:(i + 1) * L] for i in range(4)]
        wq_sb = [wq_all[:, i * 2 * DIM:(i + 1) * 2 * DIM] for i in range(4)]
        wv_sb = [wv_all[:, i * DIM:(i + 1) * DIM] for i in range(4)]
        wp_sb = [wp_all[:, i * DIM:(i + 1) * DIM] for i in range(4)]
        cos_sb = cst.tile([128, L], f16, name="t", tag="cos")
        sin_sb = cst.tile([128, L], f16, name="t", tag="sin")
        bias_sb = cst.tile([128, 8], f32, name="t", tag="bias")
        nc.sync.dma_start(cos_sb[:], cosT[:])
        nc.sync.dma_start(sin_sb[:], sinT[:])
        nc.sync.dma_start(bias_sb[:], bias[:])

        qkT = [cst.tile([64, L], f16, name="t", tag=f"qkT{m}") for m in range(16)]
        vaug = [cst.tile([128, NH * 128], f16, name="t", tag=f"vaug{i}") for i in range(8)]
        outT = [cst.tile([128, L], f16, name="t", tag=f"outT{c}") for c in range(4)]

        # ones columns of vaug never change across reps — init them once.
        for lb in range(8):
            ones_ap = vaug[lb][:].rearrange("p (h c) -> p h c", h=NH)[:, :, 64:128]
            nc.vector.memset(ones_ap, 1.0)

        def emit_body():
            # ---------- phase 1: qkv projection + RoPE ----------
            for m in range(8):
                ps = ps_b.tile([128, L], f32, name="t", tag="b")
                for kc in range(4):
                    for qb in range(2):
                        nc.tensor.matmul(
                            ps[:, qb * 512:(qb + 1) * 512],
                            wq_sb[kc][:, m * 128:(m + 1) * 128],
                            xT_sb[kc][:, qb * 512:(qb + 1) * 512],
                            start=(kc == 0), stop=(kc == 3))
                qc = sc.tile([128, L], f16, name="t", tag="qc")
                nc.vector.tensor_copy(qc[:], ps[:])
                sw = sc.tile([128, L], f16, name="t", tag="sw")
                for (do, so) in ((0, 32), (32, 0), (64, 96), (96, 64)):
                    nc.vector.tensor_copy(sw[do:do + 32, :], qc[so:so + 32, :])
                q1 = sc.tile([128, L], f16, name="t", tag="q1")
                nc.vector.tensor_mul(q1[:], qc[:], cos_sb[:])
                q2 = sc.tile([128, L], f16, name="t", tag="q2")
                nc.vector.tensor_mul(q2[:], sw[:], sin_sb[:])
                nc.vector.tensor_add(qkT[2 * m][:], q1[0:64, :], q2[0:64, :])
                nc.vector.tensor_add(qkT[2 * m + 1][:], q1[64:128, :], q2[64:128, :])

            for lb in range(8):
                vtile = ps_b.tile([128, L], f32, name="t", tag="b")
                vps = vtile[:, 0:DIM]
                for kc in range(4):
                    nc.tensor.matmul(
                        vps,
                        xT_sb[kc][:, lb * 128:(lb + 1) * 128],
                        wv_sb[kc][:],
                        start=(kc == 0), stop=(kc == 3))
                out_ap = vaug[lb][:].rearrange("p (h c) -> p h c", h=NH)[:, :, 0:64]
                in_ap = vps.rearrange("p (h c) -> p h c", h=NH)
                nc.vector.tensor_copy(out_ap, in_ap)

            # ---------- phase 2: attention (row-tiled 64x128) ----------
            def emit_scores(p):
                pts = {}
                for t in range(2):
                    h = 2 * p + t
                    QTc, KTc = qkT[h], qkT[8 + h]
                    for kb2 in range(4):
                        s = ps_s.tile([128, 2048], f32, name="t", tag="s")
                        for half in range(2):
                            kb = kb2 * 2 + half
                            for qb in range(2):
                                nc.tensor.matmul(
                                    s[:, half * 1024 + qb * 512:
                                       half * 1024 + (qb + 1) * 512],
                                    KTc[:, kb * 128:(kb + 1) * 128],
                                    QTc[:, qb * 512:(qb + 1) * 512],
                                    start=True, stop=True)
                        pt = pTp.tile([128, 2048], f16, name="t", tag="pT")
                        nc.scalar.activation(pt[:], s[:], AF.Exp,
                                             bias=bias_sb[:, 2 * kb2:2 * kb2 + 1],
                                             scale=SCALE)
                        pts[(t, kb2)] = pt
                return pts

            def emit_av_norm(p, pts):
                for t in range(2):
                    h = 2 * p + t
                    X = ps_b.tile([128, L], f32, name="t", tag="b")
                    for kc in range(8):
                        pt = pts[(t, kc // 2)]
                        off = (kc % 2) * 1024
                        va = vaug[kc][:, h * 128:(h + 1) * 128]
                        for qb in range(2):
                            q0, q1_ = off + qb * 512, off + (qb + 1) * 512
                            nc.tensor.matmul(
                                X[:, qb * 512:(qb + 1) * 512], va, pt[:, q0:q1_],
                                start=(kc == 0), stop=(kc == 7))
                    D = nrm.tile([64, L], f32, name="t", tag="D")
                    nc.vector.tensor_copy(D[:], X[64:128, :])
                    R = nrm.tile([64, L], f32, name="t", tag="R")
                    nc.vector.reciprocal_approx_fast(R[:], D[:])
                    if t == 0:
                        nc.vector.tensor_mul(outT[p][0:64, :], X[0:64, :], R[:])
                    else:
                        tmp = nrm.tile([64, L], f16, name="t", tag="tmp")
                        nc.vector.tensor_mul(tmp[:], X[0:64, :], R[:])
                        nc.sync.dma_start(outT[p][64:128, :], tmp[:])

            prev = None
            for p in range(4 if _PH >= 2 else 0):
                pts = emit_scores(p)
                if prev is not None and _PH >= 3:
                    emit_av_norm(prev[0], prev[1])
                prev = (p, pts)
            if prev is not None and _PH >= 3:
                emit_av_norm(prev[0], prev[1])

            # ---------- phase 3: output projection ----------
            yall = ysb.tile([128, 8 * DIM], f32, name="t", tag="yall")
            for lb in range(8 if _PH >= 3 else 0):
                ytile = ps_b.tile([128, L], f32, name="t", tag="b")
                yp = ytile[:, 0:DIM]
                for c in range(4):
                    nc.tensor.matmul(
                        yp,
                        outT[c][:, lb * 128:(lb + 1) * 128],
                        wp_sb[c][:],
                        start=(c == 0), stop=(c == 3))
                nc.vector.tensor_copy(yall[:, lb * DIM:(lb + 1) * DIM], yp)
            nc.sync.dma_start(
                y[:].rearrange("(lb p) d -> p lb d", p=128),
                yall[:].rearrange("p (lb d) -> p lb d", lb=8))

        # The rep loop is a hardware loop: the NEFF contains the body once,
        # and the device re-executes it _REPS times (the body is idempotent
        # — same inputs, same output locations every iteration).
        with tc.For_i(0, _REPS):
            emit_body()

    nc.compile()
    return nc


def _rope_tables():
    inv_freq = 1.0 / (10000.0 ** (np.arange(0, HD, 2, dtype=np.float32) / HD))
    t = np.arange(L, dtype=np.float32)
    freqs = np.outer(t, inv_freq)                      # (L, 32)
    emb = np.concatenate([freqs, freqs], axis=-1)      # (L, 64)
    cos = np.cos(emb).T                                # (64, L)
    sin = np.sin(emb).T                                # (64, L)
    sign = np.where(np.arange(HD) < HD // 2, -1.0, 1.0)[:, None].astype(np.float32)
    sin_s = sin * sign
    cosT = np.tile(cos, (2, 1)).astype(np.float16)     # (128, L)
    sinT = np.tile(sin_s, (2, 1)).astype(np.float16)   # (128, L)
    return cosT, sinT


_NC = None


def _get_nc():
    global _NC
    if _NC is None:
        _NC = _build_nc()
    return _NC


def _make_runner(nc):
    """Build a reusable jitted runner for `nc` (jit once, call many).

    run_bass_kernel_spmd re-traces and re-jits on every call (~1-2s of
    host overhead per call); this caches the jitted executable.
    """
    import jax
    from jax.sharding import Mesh, PartitionSpec
    from jax.experimental.shard_map import shard_map
    from concourse import bass2jax

    bass2jax.install_neuronx_cc_hook()
    partition_name = nc.partition_id_tensor.name if nc.partition_id_tensor else None
    in_names, out_names, out_avals, zero_outs = [], [], [], []
    for alloc in nc.m.functions[0].allocations:
        if not isinstance(alloc, mybir.MemoryLocationSet):
            continue
        name = alloc.memorylocations[0].name
        if alloc.kind == "ExternalInput":
            if name != partition_name:
                in_names.append(name)
        elif alloc.kind == "ExternalOutput":
            out_names.append(name)
            shape = tuple(alloc.tensor_shape)
            dtype = mybir.dt.np(alloc.dtype)
            out_avals.append(jax.core.ShapedArray(shape, dtype))
            zero_outs.append(np.zeros(shape, dtype))
    n_params = len(in_names)
    in_names_all = list(in_names) + out_names
    if partition_name is not None:
        in_names_all.append(partition_name)

    def _body(*args):
        operands = list(args)
        if partition_name is not None:
            operands.append(bass2jax.partition_id_tensor())
        outs = bass2jax._bass_exec_p.bind(
            *operands,
            out_avals=tuple(out_avals),
            in_names=tuple(in_names_all),
            out_names=tuple(out_names),
            lowering_input_output_aliases=(),
            sim_require_finite=True,
            sim_require_nnan=True,
            nc=nc,
        )
        return tuple(outs)

    devices = jax.devices()[:NCORES]
    mesh = Mesh(np.asarray(devices), ("core",))
    in_specs = (PartitionSpec("core"),) * (n_params + len(out_names))
    out_specs = (PartitionSpec("core"),) * len(out_names)
    sharded = jax.jit(
        shard_map(_body, mesh=mesh, in_specs=in_specs, out_specs=out_specs,
                  check_rep=False),
        keep_unused=True,
    )
    concat_zeros = [np.zeros((NCORES * z.shape[0], *z.shape[1:]), z.dtype)
                    for z in zero_outs]

    def run(in_maps, fetch=True):
        per_core = [[np.asarray(m[name]) for name in in_names] for m in in_maps]
        concat_in = [np.concatenate([per_core[c][i] for c in range(NCORES)], axis=0)
                     for i in range(n_params)]
        outs = sharded(*concat_in, *concat_zeros)
        if not fetch:
            jax.block_until_ready(outs)
            return None
        return [
            {name: np.asarray(outs[i]).reshape(NCORES, *out_avals[i].shape)[c]
             for i, name in enumerate(out_names)}
            for c in range(NCORES)
        ]

    return run


_RUNNER = None


def _get_runner():
    global _RUNNER
    if _RUNNER is None:
        _RUNNER = _make_runner(_get_nc())
    return _RUNNER


def _make_in_maps(x, mask, w_qkv, w_proj):
    cosT, sinT = _rope_tables()
    wq = np.ascontiguousarray(w_qkv[:, :2 * DIM]).astype(np.float16)
    wv = np.ascontiguousarray(w_qkv[:, 2 * DIM:]).astype(np.float16)
    wp = w_proj.astype(np.float16)
    in_maps = []
    for b in range(NCORES):
        xTb = np.ascontiguousarray(x[b].T).astype(np.float16)      # (512, 1024)
        bias_b = np.where(mask[b].reshape(8, 128).T, 0.0, -1e9).astype(np.float32)
        in_maps.append({
            "xT": xTb, "wq": wq, "wv": wv, "wp": wp,
            "cosT": cosT, "sinT": sinT, "bias": bias_b,
        })
    return in_maps


def kernel(x, mask, w_qkv, w_proj):
    x = np.asarray(x, dtype=np.float32)
    mask = np.asarray(mask)
    w_qkv = np.asarray(w_qkv, dtype=np.float32)
    w_proj = np.asarray(w_proj, dtype=np.float32)

    in_maps = _make_in_maps(x, mask, w_qkv, w_proj)
    run = _get_runner()
    res = run(in_maps)
    out = np.stack([res[c]["y"] for c in range(NCORES)], axis=0)
    return out.astype(np.float32)


# revision 6
# speedup vs baseline: 801.4061x; 29.6732x over previous
"""Multi-head self-attention (RoPE) Trainium2 Bass kernel.

Shards batch (B=8) across 8 NeuronCores, one batch element per core.
Per core: fused qkv projection in fp16 (with a second, column-permuted
copy of w_qk so the TensorEngine produces rotate_half(q) directly —
no partition-shuffle copies on the DVE), RoPE as two multiplies and an
add per 128-dim block, flash-style attention (per-key-block score tiles,
exp on ACT with fused scale + per-key-block mask bias, AV matmuls with a
fused ones-column producing softmax denominators), reciprocal-normalize,
and the output projection.

The repetition count (_REPS) used for timing runs as a HARDWARE loop
(tc.For_i) on-device, so timing via rep differencing measures actual
device execution rather than host-side NEFF handling.
"""
import os
import sys

# The kernel needs the 8 axon-tunneled NeuronCores visible to jax; a
# JAX_PLATFORMS=cpu pin (used by some harnesses for the reference) would
# hide them. Clear it before jax initializes through the concourse imports.
os.environ.pop("JAX_PLATFORMS", None)

sys.path.insert(0, "/opt/trn_rl_repo")

_REPS = int(os.environ.get("KREPS", "1"))
_PH = int(os.environ.get("KPHASES", "3"))

import numpy as np
from contextlib import ExitStack

import concourse.bass as bass
import concourse.tile as tile
from concourse import bacc, mybir

f32 = mybir.dt.float32
f16 = mybir.dt.float16
AF = mybir.ActivationFunctionType
ALU = mybir.AluOpType

B, L, DIM = 8, 1024, 512
NH, HD = 8, 64
SCALE = HD ** -0.5
NCORES = 8


def _build_nc():
    nc = bacc.Bacc("TRN2", target_bir_lowering=False, debug=False, enable_asserts=False)

    xT = nc.dram_tensor("xT", (DIM, L), f16, kind="ExternalInput")
    wq = nc.dram_tensor("wq", (DIM, 2 * DIM), f16, kind="ExternalInput")   # Q|K cols
    wqP = nc.dram_tensor("wqP", (DIM, 2 * DIM), f16, kind="ExternalInput") # rot-half perm
    wv = nc.dram_tensor("wv", (DIM, DIM), f16, kind="ExternalInput")       # V cols
    wp = nc.dram_tensor("wp", (DIM, DIM), f16, kind="ExternalInput")
    cosT = nc.dram_tensor("cosT", (128, L), f16, kind="ExternalInput")
    sinT = nc.dram_tensor("sinT", (128, L), f16, kind="ExternalInput")
    bias = nc.dram_tensor("bias", (128, 8), f32, kind="ExternalInput")
    y = nc.dram_tensor("y", (L, DIM), f32, kind="ExternalOutput")

    with ExitStack() as ctx:
        tc = ctx.enter_context(tile.TileContext(nc))
        cst = ctx.enter_context(tc.tile_pool(name="cst", bufs=1))
        sc = ctx.enter_context(tc.tile_pool(name="sc", bufs=3))
        pTp = ctx.enter_context(tc.tile_pool(name="pTp", bufs=18))
        nrm = ctx.enter_context(tc.tile_pool(name="nrm", bufs=2))
        ysb = ctx.enter_context(tc.tile_pool(name="ysb", bufs=1))
        # PSUM budget is 8 banks (2KB each per partition): tag "s"
        # (128x1024 f32, 2 banks, bufs=2) holds qk-projection and score
        # tiles; tag "a" (128x512 f32, 1 bank, bufs=4) holds V-projection,
        # AV-accumulation and output-projection tiles.
        ps_s = ctx.enter_context(tc.tile_pool(name="pss", bufs=2, space="PSUM"))
        ps_a = ctx.enter_context(tc.tile_pool(name="psa", bufs=4, space="PSUM"))

        # ---- load inputs ----
        xT_all = cst.tile([128, 4 * L], f16, name="t", tag="xTall")
        wq_all = cst.tile([128, 4 * 2 * DIM], f16, name="t", tag="wqall")
        wqP_all = cst.tile([128, 4 * 2 * DIM], f16, name="t", tag="wqPall")
        wv_all = cst.tile([128, 4 * DIM], f16, name="t", tag="wvall")
        wp_all = cst.tile([128, 4 * DIM], f16, name="t", tag="wpall")
        for big, dram in ((xT_all, xT), (wq_all, wq), (wqP_all, wqP),
                          (wv_all, wv), (wp_all, wp)):
            nc.sync.dma_start(
                big[:].rearrange("p (kc w) -> p kc w", kc=4),
                dram[:].rearrange("(kc p) w -> p kc w", p=128))
        xT_sb = [xT_all[:, i * L:(i + 1) * L] for i in range(4)]
        wq_sb = [wq_all[:, i * 2 * DIM:(i + 1) * 2 * DIM] for i in range(4)]
        wqP_sb = [wqP_all[:, i * 2 * DIM:(i + 1) * 2 * DIM] for i in range(4)]
        wv_sb = [wv_all[:, i * DIM:(i + 1) * DIM] for i in range(4)]
        wp_sb = [wp_all[:, i * DIM:(i + 1) * DIM] for i in range(4)]
        cos_sb = cst.tile([128, L], f16, name="t", tag="cos")
        sin_sb = cst.tile([128, L], f16, name="t", tag="sin")
        bias_sb = cst.tile([128, 8], f32, name="t", tag="bias")
        nc.sync.dma_start(cos_sb[:], cosT[:])
        nc.sync.dma_start(sin_sb[:], sinT[:])
        nc.sync.dma_start(bias_sb[:], bias[:])

        # qkT[m]: RoPE'd q/k, transposed layout (dims on partitions).
        # m 0..3 = Q dim-blocks (heads 2m, 2m+1), m 4..7 = K dim-blocks.
        qkT = [cst.tile([128, L], f16, name="t", tag=f"qkT{m}") for m in range(8)]
        vaug = [cst.tile([128, NH * 128], f16, name="t", tag=f"vaug{i}") for i in range(8)]
        outT = [cst.tile([128, L], f16, name="t", tag=f"outT{c}") for c in range(4)]

        # ones columns of vaug never change across reps — init them once.
        for lb in range(8):
            ones_ap = vaug[lb][:].rearrange("p (h c) -> p h c", h=NH)[:, :, 64:128]
            nc.vector.memset(ones_ap, 1.0)

        def emit_v(lb):
            vt = ps_a.tile([128, DIM], f32, name="t", tag="a")
            for kc in range(4):
                nc.tensor.matmul(
                    vt[:],
                    xT_sb[kc][:, lb * 128:(lb + 1) * 128],
                    wv_sb[kc][:],
                    start=(kc == 0), stop=(kc == 3))
            out_ap = vaug[lb][:].rearrange("p (h c) -> p h c", h=NH)[:, :, 0:64]
            in_ap = vt[:].rearrange("p (h c) -> p h c", h=NH)
            nc.vector.tensor_copy(out_ap, in_ap)

        def emit_qk(m):
            tq = ps_s.tile([128, L], f32, name="t", tag="s")
            for kc in range(4):
                for qb in range(2):
                    nc.tensor.matmul(
                        tq[:, qb * 512:(qb + 1) * 512],
                        wq_sb[kc][:, m * 128:(m + 1) * 128],
                        xT_sb[kc][:, qb * 512:(qb + 1) * 512],
                        start=(kc == 0), stop=(kc == 3))
            tp = ps_s.tile([128, L], f32, name="t", tag="s")
            for kc in range(4):
                for qb in range(2):
                    nc.tensor.matmul(
                        tp[:, qb * 512:(qb + 1) * 512],
                        wqP_sb[kc][:, m * 128:(m + 1) * 128],
                        xT_sb[kc][:, qb * 512:(qb + 1) * 512],
                        start=(kc == 0), stop=(kc == 3))
            qc = sc.tile([128, L], f16, name="t", tag="qc")
            nc.scalar.copy(qc[:], tq[:])
            pc = sc.tile([128, L], f16, name="t", tag="pc")
            nc.scalar.copy(pc[:], tp[:])
            for qb in range(2):
                s0, s1 = qb * 512, (qb + 1) * 512
                q1 = sc.tile([128, 512], f16, name="t", tag="q1")
                nc.vector.tensor_mul(q1[:], qc[:, s0:s1], cos_sb[:, s0:s1])
                q2 = sc.tile([128, 512], f16, name="t", tag="q2")
                nc.vector.tensor_mul(q2[:], pc[:, s0:s1], sin_sb[:, s0:s1])
                nc.vector.tensor_add(qkT[m][:, s0:s1], q1[:], q2[:])

        def emit_scores(p):
            pts = {}
            for kb in range(8):
                for t in range(2):
                    b0 = 64 * t
                    Q = qkT[p][b0:b0 + 64, :]
                    K = qkT[4 + p][b0:b0 + 64, kb * 128:(kb + 1) * 128]
                    s = ps_s.tile([128, L], f32, name="t", tag="s")
                    for qb in range(2):
                        nc.tensor.matmul(
                            s[:, qb * 512:(qb + 1) * 512],
                            K, Q[:, qb * 512:(qb + 1) * 512],
                            start=True, stop=True)
                    pt = pTp.tile([128, L], f16, name="t", tag="pT")
                    nc.scalar.activation(pt[:], s[:], AF.Exp,
                                         bias=bias_sb[:, kb:kb + 1],
                                         scale=SCALE)
                    pts[(t, kb)] = pt
            return pts

        def emit_av(p, pts):
            for t in range(2):
                h = 2 * p + t
                for qb in range(2):
                    X = ps_a.tile([128, 512], f32, name="t", tag="a")
                    for kb in range(8):
                        nc.tensor.matmul(
                            X[:],
                            vaug[kb][:, h * 128:(h + 1) * 128],
                            pts[(t, kb)][:, qb * 512:(qb + 1) * 512],
                            start=(kb == 0), stop=(kb == 7))
                    D = nrm.tile([64, 512], f32, name="t", tag="D")
                    nc.vector.tensor_copy(D[:], X[64:128, :])
                    R = nrm.tile([64, 512], f32, name="t", tag="R")
                    nc.vector.reciprocal_approx_fast(R[:], D[:])
                    s0, s1 = qb * 512, (qb + 1) * 512
                    if t == 0:
                        nc.vector.tensor_mul(outT[p][0:64, s0:s1], X[0:64, :], R[:])
                    else:
                        tmp = nrm.tile([64, 512], f16, name="t", tag="tmp")
                        nc.vector.tensor_mul(tmp[:], X[0:64, :], R[:])
                        nc.sync.dma_start(outT[p][64:128, s0:s1], tmp[:])

        def emit_proj(lb):
            yt = ps_a.tile([128, DIM], f32, name="t", tag="a")
            for c in range(4):
                nc.tensor.matmul(
                    yt[:],
                    outT[c][:, lb * 128:(lb + 1) * 128],
                    wp_sb[c][:],
                    start=(c == 0), stop=(c == 3))
            nc.vector.tensor_copy(yall[:, lb * DIM:(lb + 1) * DIM], yt[:])

        def emit_body():
            for lb in range(8):
                emit_v(lb)
            prev = None
            for p in range(4 if _PH >= 2 else 0):
                emit_qk(p)
                emit_qk(4 + p)
                if prev is not None and _PH >= 3:
                    emit_av(prev[0], prev[1])
                pts = emit_scores(p)
                prev = (p, pts)
            if prev is not None and _PH >= 3:
                emit_av(prev[0], prev[1])
            for lb in range(8 if _PH >= 3 else 0):
                emit_proj(lb)
            nc.sync.dma_start(
                y[:].rearrange("(lb p) d -> p lb d", p=128),
                yall[:].rearrange("p (lb d) -> p lb d", lb=8))

        with tc.For_i(0, _REPS):
            yall = ysb.tile([128, 8 * DIM], f32, name="t", tag="yall")
            emit_body()

    nc.compile()
    return nc


def _rope_tables():
    inv_freq = 1.0 / (10000.0 ** (np.arange(0, HD, 2, dtype=np.float32) / HD))
    t = np.arange(L, dtype=np.float32)
    freqs = np.outer(t, inv_freq)                      # (L, 32)
    emb = np.concatenate([freqs, freqs], axis=-1)      # (L, 64)
    cos = np.cos(emb).T                                # (64, L)
    sin = np.sin(emb).T                                # (64, L)
    sign = np.where(np.arange(HD) < HD // 2, -1.0, 1.0)[:, None].astype(np.float32)
    sin_s = sin * sign
    cosT = np.tile(cos, (2, 1)).astype(np.float16)     # (128, L)
    sinT = np.tile(sin_s, (2, 1)).astype(np.float16)   # (128, L)
    return cosT, sinT


_NC = None


def _get_nc():
    global _NC
    if _NC is None:
        _NC = _build_nc()
    return _NC


def _make_runner(nc):
    """Build a reusable jitted runner for `nc` (jit once, call many).

    run_bass_kernel_spmd re-traces and re-jits on every call (~1-2s of
    host overhead per call); this caches the jitted executable.
    """
    import jax
    from jax.sharding import Mesh, PartitionSpec
    from jax.experimental.shard_map import shard_map
    from concourse import bass2jax

    bass2jax.install_neuronx_cc_hook()
    partition_name = nc.partition_id_tensor.name if nc.partition_id_tensor else None
    in_names, out_names, out_avals, zero_outs = [], [], [], []
    for alloc in nc.m.functions[0].allocations:
        if not isinstance(alloc, mybir.MemoryLocationSet):
            continue
        name = alloc.memorylocations[0].name
        if alloc.kind == "ExternalInput":
            if name != partition_name:
                in_names.append(name)
        elif alloc.kind == "ExternalOutput":
            out_names.append(name)
            shape = tuple(alloc.tensor_shape)
            dtype = mybir.dt.np(alloc.dtype)
            out_avals.append(jax.core.ShapedArray(shape, dtype))
            zero_outs.append(np.zeros(shape, dtype))
    n_params = len(in_names)
    in_names_all = list(in_names) + out_names
    if partition_name is not None:
        in_names_all.append(partition_name)

    def _body(*args):
        operands = list(args)
        if partition_name is not None:
            operands.append(bass2jax.partition_id_tensor())
        outs = bass2jax._bass_exec_p.bind(
            *operands,
            out_avals=tuple(out_avals),
            in_names=tuple(in_names_all),
            out_names=tuple(out_names),
            lowering_input_output_aliases=(),
            sim_require_finite=True,
            sim_require_nnan=True,
            nc=nc,
        )
        return tuple(outs)

    devices = jax.devices()[:NCORES]
    mesh = Mesh(np.asarray(devices), ("core",))
    in_specs = (PartitionSpec("core"),) * (n_params + len(out_names))
    out_specs = (PartitionSpec("core"),) * len(out_names)
    sharded = jax.jit(
        shard_map(_body, mesh=mesh, in_specs=in_specs, out_specs=out_specs,
                  check_rep=False),
        keep_unused=True,
    )
    concat_zeros = [np.zeros((NCORES * z.shape[0], *z.shape[1:]), z.dtype)
                    for z in zero_outs]

    def prepare(in_maps):
        """Device-put the concatenated inputs once; reusable across calls."""
        per_core = [[np.asarray(m[name]) for name in in_names] for m in in_maps]
        concat_in = [np.concatenate([per_core[c][i] for c in range(NCORES)], axis=0)
                     for i in range(n_params)]
        return ([jax.device_put(a) for a in concat_in],
                [jax.device_put(z) for z in concat_zeros])

    def run_prepared(dev, fetch=True):
        dev_in, dev_zeros = dev
        outs = sharded(*dev_in, *dev_zeros)
        if not fetch:
            jax.block_until_ready(outs)
            return None
        return [
            {name: np.asarray(outs[i]).reshape(NCORES, *out_avals[i].shape)[c]
             for i, name in enumerate(out_names)}
            for c in range(NCORES)
        ]

    def run(in_maps, fetch=True):
        return run_prepared(prepare(in_maps), fetch=fetch)

    run.prepare = prepare
    run.run_prepared = run_prepared
    return run


_RUNNER = None


def _get_runner():
    global _RUNNER
    if _RUNNER is None:
        _RUNNER = _make_runner(_get_nc())
    return _RUNNER


def _make_in_maps(x, mask, w_qkv, w_proj):
    cosT, sinT = _rope_tables()
    wq = np.ascontiguousarray(w_qkv[:, :2 * DIM]).astype(np.float16)
    # rotate_half as a column permutation: wqP[:, d] = wq[:, d xor 32]
    # (within each head's 64-column group); the sign lives in sinT.
    perm = (np.arange(2 * DIM) // HD) * HD + ((np.arange(2 * DIM) % HD) ^ 32)
    wqP = np.ascontiguousarray(wq[:, perm])
    wv = np.ascontiguousarray(w_qkv[:, 2 * DIM:]).astype(np.float16)
    wp = w_proj.astype(np.float16)
    in_maps = []
    for b in range(NCORES):
        xTb = np.ascontiguousarray(x[b].T).astype(np.float16)      # (512, 1024)
        bias_b = np.where(mask[b].reshape(8, 128).T, 0.0, -1e9).astype(np.float32)
        in_maps.append({
            "xT": xTb, "wq": wq, "wqP": wqP, "wv": wv, "wp": wp,
            "cosT": cosT, "sinT": sinT, "bias": bias_b,
        })
    return in_maps


def kernel(x, mask, w_qkv, w_proj):
    x = np.asarray(x, dtype=np.float32)
    mask = np.asarray(mask)
    w_qkv = np.asarray(w_qkv, dtype=np.float32)
    w_proj = np.asarray(w_proj, dtype=np.float32)

    in_maps = _make_in_maps(x, mask, w_qkv, w_proj)
    run = _get_runner()
    res = run(in_maps)
    out = np.stack([res[c]["y"] for c in range(NCORES)], axis=0)
    return out.astype(np.float32)
